# revision 34
# baseline (speedup 1.0000x reference)
"""Trainium2 Bass/Tile kernel for nn_MultiHeadHomogeneousAttention.

Sharding: 8 cores = 4 batches x 2 query-sequence halves. Every core runs the
identical SPMD program on its own data slice:
  - computes K/V causal-conv projections for all 8 heads of its batch over the
    full sequence, and the Q projection for its query half,
  - flash-style attention entirely in transposed [feature, seq] layout,
  - output projection + residual + LayerNorm for its half,
  - writes a disjoint (1024, 1024) fp32 output shard; host concatenates.

Numerics: conv/Q/PV/out-projection matmuls run in fp8e4m3 with the DoubleRow
perf mode (two 128-deep contraction tiles per instruction) accumulating in
fp32 PSUM; weights are pre-scaled x64 on host to dodge fp8 subnormals and
un-scaled at PSUM evacuation. Scores q.k stay bf16 (contraction is P=128, no
pairing possible). Softmax without max-subtraction (scores bounded ~|4.5|);
probabilities stored fp8; the softmax denominator is computed by a fp8
ones-matmul over the same fp8 probabilities, so normalization is exact w.r.t.
the quantized weights. bk dropped (softmax shift invariance); bv and bo folded
into the residual on host; residual/LayerNorm fp32. Measured end-to-end error
vs fp32 reference: ~1.2e-3 relative.

Heads are processed in kernel-size-sorted order (PERM) so the tap loops are
uniform across cores; Wo columns are permuted to match so the output needs no
unpermutation.

Schedule: V conv runs first (PE-bound, Act does fp8 evacuations); then the
attention c=0 windows are PE-bound with slot s+1's K-conv/Q-proj chunks woven
between the score/exp steps of window s (evacuated on DVE since Act is
exp-saturated); the c=1 windows are Act(exp)-bound with the output
projection+LayerNorm for the first half of rows interleaved, its heavy
normalize part deferred two windows so PSUM recycles off the DVE queue head.
Scores pipeline through 2x[128,2,512] PSUM tiles (one 1024-wide exp per
sk-pair); the softmax-denominator ones-matmul uses a 128-wide all-ones lhsT so
every PSUM partition row holds l and the reciprocal needs no partition
broadcast. dual-fp8 ISA restrictions handled: 16B-aligned outer steps
(2064-col padded key/value rows, 128-wide ones tile) and 2B-aligned moving
starts (a 1-column-shifted second copy of keyT serves the odd conv tap).

TimelineSim: 231.8us vs 470.9us for the bf16 baseline (2.03x).
"""

import sys

sys.path.insert(0, "/opt/trn_rl_repo")

import numpy as np
import ml_dtypes
from contextlib import ExitStack

BF16 = ml_dtypes.bfloat16
F8 = ml_dtypes.float8_e4m3

# ---- problem constants (hardcoded; harness provides matching inputs) ----
B = 4
S = 2048
D = 1024          # dim_m
P = 128           # dim_proj
H = 8
KMAX = 3
LN_EPS = 1e-12
KSIZES = (1, 1, 1, 2, 2, 3, 3, 3)        # per original head index
PERM = (5, 6, 7, 3, 4, 0, 1, 2)          # slot -> original head (ksize desc)
SLOT_K = tuple(KSIZES[h] for h in PERM)  # (3,3,3,2,2,1,1,1)

# K-conv (slot, tap) pairs, slot-major, tap descending (t=KMAX-1 first)
KT_PAIRS = [(s, t) for s in range(H)
            for t in range(KMAX - 1, KMAX - 1 - SLOT_K[s], -1)]
# V-conv moving-weight blocks, tap-major: t=2 slots 0..7, t=1 slots 0..4, t=0 slots 0..2
VT_BLOCKS = [(t, s) for t in range(KMAX - 1, -1, -1)
             for s in range(H) if SLOT_K[s] >= KMAX - t]
NKT = len(KT_PAIRS)   # 16
NVT = len(VT_BLOCKS)  # 16

N_CORES = 8
HALF = S // 2
CH = 512              # free-dim chunk width (one PSUM bank of fp32)
NDT = D // 128        # 8 d-tiles
NDP = NDT // 2        # 4 d-tile pairs (fp8 DoubleRow contraction pairs)
NSK = S // 128        # 16 key-side seq tiles
NSP = NSK // 2        # 8 key-side seq-tile pairs
NST = HALF // 128     # 8 output seq tiles
WSCALE = 64.0         # fp8 weight pre-scale
WINV = float(1.0 / WSCALE)


def _emit(tc, io):
    """Emit the per-core Tile program. io: dict of DRAM APs."""
    from concourse import mybir

    nc = tc.nc
    f32 = mybir.dt.float32
    bf16 = mybir.dt.bfloat16
    f8 = mybir.dt.float8e4
    AF = mybir.ActivationFunctionType
    ALU = mybir.AluOpType
    DR = mybir.MatmulPerfMode.DoubleRow

    ctx = ExitStack()
    with ctx:
        # ---------------- pools ----------------
        vTp = ctx.enter_context(tc.tile_pool(name="vTp", bufs=NDP))
        wvp = ctx.enter_context(tc.tile_pool(name="wvp", bufs=NDP))
        kTp = ctx.enter_context(tc.tile_pool(name="kTp", bufs=2 * NDP))
        wkp = ctx.enter_context(tc.tile_pool(name="wkp", bufs=NDP))
        qTp = ctx.enter_context(tc.tile_pool(name="qTp", bufs=NDP))
        wqp = ctx.enter_context(tc.tile_pool(name="wqp", bufs=NDP))
        kts = ctx.enter_context(tc.tile_pool(name="kts", bufs=H))
        vs = ctx.enter_context(tc.tile_pool(name="vs", bufs=NSP))
        qts = ctx.enter_context(tc.tile_pool(name="qts", bufs=H))
        ctxn = ctx.enter_context(tc.tile_pool(name="ctxn", bufs=H // 2))
        wop = ctx.enter_context(tc.tile_pool(name="wop", bufs=H // 2))
        ptp = ctx.enter_context(tc.tile_pool(name="ptp", bufs=6))
        rbp = ctx.enter_context(tc.tile_pool(name="rbp", bufs=3))
        resp = ctx.enter_context(tc.tile_pool(name="resp", bufs=3))
        hbp = ctx.enter_context(tc.tile_pool(name="hbp", bufs=3))
        smalls = ctx.enter_context(tc.tile_pool(name="smalls", bufs=1))
        lnp = ctx.enter_context(tc.tile_pool(name="lnp", bufs=3))
        psC = ctx.enter_context(tc.tile_pool(name="psC", bufs=2, space="PSUM"))
        psK = ctx.enter_context(tc.tile_pool(name="psK", bufs=2, space="PSUM"))
        psA = ctx.enter_context(tc.tile_pool(name="psA", bufs=1, space="PSUM"))
        psL = ctx.enter_context(tc.tile_pool(name="psL", bufs=1, space="PSUM"))

        # ---------------- constants ----------------
        bq_t = smalls.tile([128, H], f32, tag="bq")
        nc.sync.dma_start(out=bq_t, in_=io["bq"][:, :])
        gamma_t = smalls.tile([128, D], bf16, tag="gamma")
        nc.sync.dma_start(out=gamma_t, in_=io["gamma"][:, :])
        beta_t = smalls.tile([128, D], bf16, tag="beta")
        nc.sync.dma_start(out=beta_t, in_=io["beta"][:, :])
        eps_t = smalls.tile([128, 1], f32, tag="eps")
        nc.vector.memset(eps_t, LN_EPS)
        ones_t = smalls.tile([128, 2, 128], f8, tag="ones")
        nc.vector.memset(ones_t, 1.0)

        # ---------------- phase 1: V causal conv -> V_s (fp8, sk pairs) -----
        valT = [vTp.tile([128, 2, S + 16], f8, tag="vT", name="vTt")
                for _ in range(NDP)]
        WvT = [wvp.tile([128, 2, NVT * 128], f8, tag="wv", name="wvt")
               for _ in range(NDP)]
        for p in range(NDP):
            nc.sync.dma_start(out=WvT[p], in_=io["Wvt"][p])
        for p in range(NDP):
            nc.vector.memset(valT[p][:, :, 0:2], 0.0)
            nc.sync.dma_start(out=valT[p][:, :, 2:CH + 2],
                              in_=io["vT"][p][:, :, 0:CH])
        for p in range(NDP):
            nc.sync.dma_start(out=valT[p][:, :, CH + 2:S + 2],
                              in_=io["vT"][p][:, :, CH:S])

        # moving-block layout: per (tap, half-group) contiguous runs
        def vt_runs(hg):
            lo_s, hi_s = hg * 4, hg * 4 + 4
            runs = []
            for t in range(KMAX - 1, -1, -1):
                blks = [i for i, (tt, s) in enumerate(VT_BLOCKS)
                        if tt == t and lo_s <= s < hi_s]
                if blks:
                    s0 = VT_BLOCKS[blks[0]][1]
                    runs.append((t, blks[0] * 128, len(blks) * 128,
                                 (s0 - lo_s) * 128))
            return runs  # (tap, w_col_off, width, psum_col_off)

        V_s = [vs.tile([128, 2, H * 128], f8, tag="vs", name="vst")
               for _ in range(NSP)]
        for sk in range(NSK):
            ps = psC.tile([128, 2, CH], f32, tag="mm")
            for hg in range(2):
                mms = [(ps[:, hg, pof:pof + wid],
                        valT[p][:, :, sk * 128 + t:sk * 128 + t + 128],
                        WvT[p][:, :, wof:wof + wid])
                       for p in range(NDP)
                       for (t, wof, wid, pof) in vt_runs(hg)]
                n = len(mms)
                for i, (o, l, r) in enumerate(mms):
                    nc.tensor.matmul(o, lhsT=l, rhs=r, start=(i == 0),
                                     stop=(i == n - 1), perf_mode=DR,
                                     skip_group_check=True)
            nc.scalar.activation(out=V_s[sk // 2][:, sk % 2, :], in_=ps,
                                 func=AF.Copy, scale=WINV)

        # ---------------- phase 2/3 emitters: K conv + Q proj per slot ------
        keyT = [kTp.tile([128, 2, S + 16], f8, tag="kT", name="kTt")
                for _ in range(NDP)]
        keyB = [kTp.tile([128, 2, S + 16], f8, tag="kT", name="kTt")
                for _ in range(NDP)]
        WkT = [wkp.tile([128, 2, NKT * 128], f8, tag="wk", name="wkt")
               for _ in range(NDP)]
        for p in range(NDP):
            nc.vector.memset(keyT[p][:, :, 0:2], 0.0)
            nc.sync.dma_start(out=keyT[p][:, :, 2:S + 2], in_=io["kT"][p])
            nc.vector.memset(keyB[p][:, :, 0:1], 0.0)
            nc.sync.dma_start(out=keyB[p][:, :, 1:S + 1], in_=io["kT"][p])
            nc.sync.dma_start(out=WkT[p], in_=io["Wkt"][p])

        qT_in = [qTp.tile([128, 2, HALF], f8, tag="qT", name="qTt")
                 for _ in range(NDP)]
        WqT = [wqp.tile([128, 2, H * 128], f8, tag="wq", name="wqt")
               for _ in range(NDP)]
        for p in range(NDP):
            nc.sync.dma_start(out=qT_in[p], in_=io["qT"][p])
            nc.sync.dma_start(out=WqT[p], in_=io["Wqt"][p])

        kT_s = [kts.tile([128, S], bf16, tag="kts", name="ktst")
                for _ in range(H)]
        qT_s = [qts.tile([128, HALF], bf16, tag="qts", name="qtst")
                for _ in range(H)]

        def k_chunk(slot, cq, on_act):
            pairs = [(i, t) for i, (s, t) in enumerate(KT_PAIRS) if s == slot]
            c0 = cq * CH
            ps = psK.tile([128, CH], f32, tag="kq")
            mms = [(ps[:, :],
                    WkT[p][:, :, i * 128:(i + 1) * 128],
                    keyT[p][:, :, c0 + t:c0 + t + CH] if t % 2 == 0
                    else keyB[p][:, :, c0 + t - 1:c0 + t - 1 + CH])
                   for p in range(NDP) for i, t in pairs]
            n = len(mms)
            for i2, (o, l, r) in enumerate(mms):
                nc.tensor.matmul(o, lhsT=l, rhs=r, start=(i2 == 0),
                                 stop=(i2 == n - 1), perf_mode=DR,
                                 skip_group_check=True)
            dst = kT_s[slot][:, c0:c0 + CH]
            if on_act:
                nc.scalar.activation(out=dst, in_=ps, func=AF.Copy,
                                     scale=WINV)
            else:
                nc.vector.tensor_scalar_mul(dst, ps, WINV)

        def q_chunk(slot, half, on_act):
            ps = psK.tile([128, CH], f32, tag="kq")
            mms = [(ps[:, :],
                    WqT[p][:, :, slot * 128:(slot + 1) * 128],
                    qT_in[p][:, :, half * CH:(half + 1) * CH])
                   for p in range(NDP)]
            n = len(mms)
            for i, (o, l, r) in enumerate(mms):
                nc.tensor.matmul(o, lhsT=l, rhs=r, start=(i == 0),
                                 stop=(i == n - 1), perf_mode=DR,
                                 skip_group_check=True)
            dst = qT_s[slot][:, half * CH:(half + 1) * CH]
            if on_act:
                nc.scalar.activation(out=dst, in_=ps, func=AF.Identity,
                                     bias=bq_t[:, slot:slot + 1], scale=WINV)
            else:
                nc.vector.tensor_scalar(out=dst, in0=ps, scalar1=WINV,
                                        scalar2=bq_t[:, slot:slot + 1],
                                        op0=ALU.mult, op1=ALU.add)

        def kq_fillers(slot, on_act=False):
            return ([lambda cq=cq: k_chunk(slot, cq, on_act)
                     for cq in range(4)] +
                    [lambda hf=hf: q_chunk(slot, hf, on_act)
                     for hf in range(2)])

        # ---------------- phase 4: attention (transposed flash) -------------
        ctxN = [ctxn.tile([128, 2, HALF], f8, tag="ctxn", name="ctxnt")
                for _ in range(H // 2)]

        def att(c, slot, fillers=()):
            fillers = list(fillers)
            ctx_ps = psA.tile([128, CH], f32, tag="ctx")
            l_ps = psL.tile([128, CH], f32, tag="l")

            def sc(skp):
                ps = psC.tile([128, 2, CH], f32, tag="mm")
                for j in range(2):
                    sk = skp * 2 + j
                    nc.tensor.matmul(
                        ps[:, j, :],
                        lhsT=kT_s[slot][:, sk * 128:(sk + 1) * 128],
                        rhs=qT_s[slot][:, c * CH:(c + 1) * CH],
                        start=True, stop=True, skip_group_check=True)
                return ps

            def consume(skp, ps):
                pt = ptp.tile([128, 2, CH], f8, tag="pt")
                nc.scalar.activation(out=pt, in_=ps, func=AF.Exp)
                nc.tensor.matmul(
                    ctx_ps[:, :],
                    lhsT=V_s[skp][:, :, slot * 128:(slot + 1) * 128],
                    rhs=pt[:, :, :],
                    start=(skp == 0), stop=(skp == NSP - 1),
                    perf_mode=DR, skip_group_check=True)
                # ones lhsT is 128 wide: every PSUM partition row gets l,
                # so the reciprocal below needs no partition broadcast
                nc.tensor.matmul(
                    l_ps[:, :], lhsT=ones_t[:, :, :], rhs=pt[:, :, :],
                    start=(skp == 0), stop=(skp == NSP - 1),
                    perf_mode=DR, skip_group_check=True)

            prev = sc(0)
            for skp in range(1, NSP):
                cur = sc(skp)
                consume(skp - 1, prev)
                if fillers:
                    fillers.pop(0)()
                prev = cur
            consume(NSP - 1, prev)
            for f in fillers:
                f()

            rb_t = rbp.tile([128, CH], f32, tag="rb")
            nc.vector.reciprocal(out=rb_t, in_=l_ps)
            nc.vector.tensor_mul(
                out=ctxN[slot // 2][:, slot % 2, c * CH:(c + 1) * CH],
                in0=ctx_ps, in1=rb_t)

        # ---------------- phase 5 emitter: out proj + residual + LN ---------
        WoT = [wop.tile([128, 2, D], f8, tag="wop", name="wopt")
               for _ in range(H // 2)]
        for sp in range(H // 2):
            nc.sync.dma_start(out=WoT[sp], in_=io["Wot"][sp])

        def out_p1(st):
            """Out-proj + residual + LN stats for one seq tile. The heavy
            normalize half is deferred (out_p2) so the stt evacs here stay
            near the DVE queue head and recycle psK promptly."""
            res_t = resp.tile([128, D], f32, tag="res")
            nc.sync.dma_start(out=res_t,
                              in_=io["res"][st * 128:(st + 1) * 128, :])
            h_t = hbp.tile([128, D], bf16, tag="hb")
            for mc in range(2):
                ps = psK.tile([128, CH], f32, tag="kq")
                for sp in range(H // 2):
                    nc.tensor.matmul(
                        ps[:, :],
                        lhsT=ctxN[sp][:, :, st * 128:(st + 1) * 128],
                        rhs=WoT[sp][:, :, mc * CH:(mc + 1) * CH],
                        start=(sp == 0), stop=(sp == H // 2 - 1),
                        perf_mode=DR, skip_group_check=True)
                nc.vector.scalar_tensor_tensor(
                    out=h_t[:, mc * CH:(mc + 1) * CH], in0=ps, scalar=WINV,
                    in1=res_t[:, mc * CH:(mc + 1) * CH],
                    op0=ALU.mult, op1=ALU.add)
            stats = lnp.tile([128, 2, 6], f32, tag="stats")
            for sub in range(2):
                nc.vector.bn_stats(out=stats[:, sub, :],
                                   in_=h_t[:, sub * CH:(sub + 1) * CH])
            mv = lnp.tile([128, 2], f32, tag="mv")
            nc.vector.bn_aggr(out=mv, in_=stats)
            std = lnp.tile([128, 1], f32, tag="std")
            nc.scalar.activation(out=std, in_=mv[:, 1:2], func=AF.Sqrt,
                                 bias=eps_t[:, :], scale=1.0)
            rstd = lnp.tile([128, 1], f32, tag="rstd")
            nc.vector.reciprocal(out=rstd, in_=std)
            return (st, h_t, mv, rstd)

        def out_p2(state, tail=False):
            st, h_t, mv, rstd = state
            if tail:
                # Act/Pool are idle after the last exp: normalize there.
                nmr = lnp.tile([128, 1], f32, tag="nmr")
                nc.vector.scalar_tensor_tensor(
                    out=nmr, in0=mv[:, 0:1], scalar=-1.0, in1=rstd,
                    op0=ALU.mult, op1=ALU.mult)
                nc.scalar.activation(out=h_t, in_=h_t, func=AF.Identity,
                                     bias=nmr, scale=rstd)
                nc.vector.tensor_mul(out=h_t[:, :], in0=h_t[:, :],
                                     in1=gamma_t)
            else:
                nc.vector.tensor_scalar(
                    out=h_t[:, :], in0=h_t[:, :],
                    scalar1=mv[:, 0:1], scalar2=rstd,
                    op0=ALU.subtract, op1=ALU.mult)
                nc.vector.tensor_mul(out=h_t[:, :], in0=h_t[:, :],
                                     in1=gamma_t)
            nc.gpsimd.tensor_add(out=h_t[:, :], in0=h_t[:, :], in1=beta_t)
            nc.sync.dma_start(out=io["out"][st * 128:(st + 1) * 128, :],
                              in_=h_t)

        # ---------------- emission schedule ----------------
        # slot 0's K/Q up front on Act; slot s+1's conv chunks are woven
        # between the score/exp steps of attention window s (DVE evac there,
        # Act is exp-bound).
        for f in kq_fillers(0, on_act=True):
            f()
        for slot in range(H):
            fillers = kq_fillers(slot + 1) if slot + 1 < H else ()
            att(0, slot, fillers)
        pend = []
        for slot in range(H):
            att(1, slot)
            if slot < NST // 2:
                pend.append(out_p1(slot))    # st 0-3: c=0 columns only
            if slot >= 2 and pend:
                out_p2(pend.pop(0))
        for st in range(NST // 2, NST):
            pend.append(out_p1(st))
            if len(pend) > 1:
                out_p2(pend.pop(0))
        while pend:
            out_p2(pend.pop(0))


# ---------------------------------------------------------------------------
# host-side build / prep / run
# ---------------------------------------------------------------------------
_CACHE = {}


def _build():
    import concourse.tile as tile
    from concourse import bacc, mybir

    nc = bacc.Bacc("TRN2", target_bir_lowering=False, debug=False,
                   enable_asserts=False, num_devices=N_CORES,
                   dynamic_dma_scratch_size=4096)
    f32 = mybir.dt.float32
    bf16 = mybir.dt.bfloat16
    f8 = mybir.dt.float8e4
    io = {
        "kT": nc.dram_tensor("kT", [NDP, 128, 2, S], f8, kind="ExternalInput").ap(),
        "vT": nc.dram_tensor("vT", [NDP, 128, 2, S], f8, kind="ExternalInput").ap(),
        "qT": nc.dram_tensor("qT", [NDP, 128, 2, HALF], f8, kind="ExternalInput").ap(),
        "res": nc.dram_tensor("res", [HALF, D], f32, kind="ExternalInput").ap(),
        "Wkt": nc.dram_tensor("Wkt", [NDP, 128, 2, NKT * 128], f8, kind="ExternalInput").ap(),
        "Wvt": nc.dram_tensor("Wvt", [NDP, 128, 2, NVT * 128], f8, kind="ExternalInput").ap(),
        "Wqt": nc.dram_tensor("Wqt", [NDP, 128, 2, H * 128], f8, kind="ExternalInput").ap(),
        "Wot": nc.dram_tensor("Wot", [H // 2, 128, 2, D], f8, kind="ExternalInput").ap(),
        "bq": nc.dram_tensor("bq", [128, H], f32, kind="ExternalInput").ap(),
        "gamma": nc.dram_tensor("gamma", [128, D], bf16, kind="ExternalInput").ap(),
        "beta": nc.dram_tensor("beta", [128, D], bf16, kind="ExternalInput").ap(),
        "out": nc.dram_tensor("out", [HALF, D], bf16, kind="ExternalOutput").ap(),
    }
    with tile.TileContext(nc) as tc:
        _emit(tc, io)
    nc.compile()
    return nc


def _pairT(x, n):
    """[D, N] fp32 -> [NDP, 128, 2, N] fp8 d-tile-pair layout."""
    return np.ascontiguousarray(
        x.reshape(NDP, 2, 128, n).transpose(0, 2, 1, 3)).astype(F8)


def _prep_weights(Wq, bq, Wk, Wv, Wo, bo, bv, gamma, beta):
    """Shared (all-core) weight tensors, permuted + scaled + cast to fp8."""
    scale = np.float32(P ** -0.5)

    WkTf = Wk.transpose(0, 2, 1, 3)  # (H, D, P, K)
    Wkt = np.empty((NDP, 128, 2, NKT * 128), np.float32)
    for i, (slot, t) in enumerate(KT_PAIRS):
        blk = (WkTf[PERM[slot], :, :, t] * WSCALE).reshape(NDP, 2, 128, P)
        for j in range(2):
            Wkt[:, :, j, i * 128:(i + 1) * 128] = blk[:, j]

    WvTf = Wv.transpose(0, 2, 1, 3)
    Wvt = np.empty((NDP, 128, 2, NVT * 128), np.float32)
    for i, (t, slot) in enumerate(VT_BLOCKS):
        blk = (WvTf[PERM[slot], :, :, t] * WSCALE).reshape(NDP, 2, 128, P)
        for j in range(2):
            Wvt[:, :, j, i * 128:(i + 1) * 128] = blk[:, j]

    WqTf = Wq.transpose(0, 2, 1) * (scale * WSCALE)  # (H, D, P)
    Wqt = np.empty((NDP, 128, 2, H * 128), np.float32)
    for slot in range(H):
        blk = WqTf[PERM[slot]].reshape(NDP, 2, 128, P)
        for j in range(2):
            Wqt[:, :, j, slot * 128:(slot + 1) * 128] = blk[:, j]

    Wot = np.empty((H // 2, 128, 2, D), np.float32)
    for slot in range(H):
        hp = PERM[slot]
        Wot[slot // 2, :, slot % 2, :] = Wo[:, hp * P:(hp + 1) * P].T * WSCALE

    bq_t = np.empty((128, H), np.float32)
    for slot in range(H):
        bq_t[:, slot] = bq[PERM[slot]] * scale

    # bv folded into residual constant: sum_h bv_h @ Wo_cols_h  (+ bo)
    bv_fold = np.einsum("hp,mhp->m", bv, Wo.reshape(D, H, P)).astype(np.float32)
    res_const = (bo + bv_fold).astype(np.float32)

    return {
        "Wkt": Wkt.astype(F8), "Wvt": Wvt.astype(F8),
        "Wqt": Wqt.astype(F8), "Wot": Wot.astype(F8),
        "bq": bq_t,
        "gamma": np.broadcast_to(gamma, (128, D)).astype(BF16).copy(),
        "beta": np.broadcast_to(beta, (128, D)).astype(BF16).copy(),
    }, res_const


def _prep_core(query, key, value, res_const, b, j):
    """Per-core activation tensors for core (b, j)."""
    kT = _pairT(np.ascontiguousarray(key[b].T), S)
    vT = _pairT(np.ascontiguousarray(value[b].T), S)
    qh = query[b, j * HALF:(j + 1) * HALF, :]
    qT = _pairT(np.ascontiguousarray(query[b].T[:, j * HALF:(j + 1) * HALF]),
                HALF)
    res = (qh + res_const).astype(np.float32)
    return {"kT": kT, "vT": vT, "qT": qT, "res": res}


def kernel(value, key, query, Wq, bq, Wk, bk, Wv, bv, Wo, bo, gamma, beta):
    from concourse.bass_utils import run_bass_kernel_spmd

    value = np.asarray(value, np.float32)
    key = np.asarray(key, np.float32)
    query = np.asarray(query, np.float32)
    Wq = np.asarray(Wq, np.float32)
    bq = np.asarray(bq, np.float32)
    Wk = np.asarray(Wk, np.float32)
    Wv = np.asarray(Wv, np.float32)
    bv = np.asarray(bv, np.float32)
    Wo = np.asarray(Wo, np.float32)
    bo = np.asarray(bo, np.float32)
    gamma = np.asarray(gamma, np.float32)
    beta = np.asarray(beta, np.float32)

    if "nc" not in _CACHE:
        _CACHE["nc"] = _build()
    nc = _CACHE["nc"]

    wmaps, res_const = _prep_weights(Wq, bq, Wk, Wv, Wo, bo, bv, gamma, beta)
    in_maps = []
    for core in range(N_CORES):
        b, j = divmod(core, 2)
        m = dict(wmaps)
        m.update(_prep_core(query, key, value, res_const, b, j))
        in_maps.append(m)

    trace = _CACHE.get("trace", False)
    rr = run_bass_kernel_spmd(nc, in_maps, core_ids=list(range(N_CORES)),
                              trace=trace)
    if trace:
        _CACHE["last_results"] = rr

    out = np.empty((B, S, D), np.float32)
    for core in range(N_CORES):
        b, j = divmod(core, 2)
        out[b, j * HALF:(j + 1) * HALF, :] = \
            np.asarray(rr.results[core]["out"]).astype(np.float32)
    return out
